# revision 12
# baseline (speedup 1.0000x reference)
"""Distributed Trainium2 kernel for nn_Attention (B=2, N=2048, D=768, H=12).

Sharding: core c handles batch c//4 and head-triple c%4 (3 heads) for the
attention; the FC output projection is query-split (core c computes rows
(c%4)*512 .. +512 of its batch). The context tensor moves between the two
shardings with per-head 8-core AllToAlls of ctx^T column blocks; the FC
contracts over all 8 ranks' head-dims with rows of w_fc^T zeroed for the
other batch's ranks, which keeps the SPMD program identical on every core.

Compute dtype: bf16 matmul operands, fp32 PSUM accumulation and softmax.
The padding mask is folded into the exp bias (-1e7 per masked key), the
1/sqrt(hd) scale into the pre-transposed q weights, and the softmax
denominator rides the PV matmul as a 65th ones-column of V.
"""

import sys
import numpy as np

sys.path.insert(0, "/opt/trn_rl_repo")

import ml_dtypes

B, N, D, H, HD = 2, 2048, 768, 12, 64
P = 128
NCORES = 8
HPC = 3  # heads per core
NC_I4 = N // 512
NC_KC = N // P
NC_CC = D // P
SCALE = HD ** (-0.5)
MASK_VAL = -10000000.0

_BF16 = ml_dtypes.bfloat16


def _fix_multi_waits(nc):
    """walrus in this container accepts only ONE semaphore wait per
    instruction; hoist extra waits onto EventSemaphore carriers inserted
    immediately before, on the same engine (program order preserved)."""
    import bass_rust

    for b in nc.main_func.blocks:
        insts = b.instructions
        idx = 0
        while idx < len(insts):
            ins = insts[idx]
            si = ins.sync_info
            if si is None or len(si.on_wait) <= 1:
                idx += 1
                continue
            waits = list(si.on_wait)
            excess, keep = waits[:-1], waits[-1:]
            carriers = []
            for k, w in enumerate(excess):
                e = bass_rust.InstEventSemaphore(
                    name=f"{ins.name}_waitsplit_{k}", ins=[], outs=[]
                )
                e.engine = ins.engine
                esi = e.sync_info
                if esi is None:
                    esi = bass_rust.SyncInfo(on_wait=[], on_update=[])
                esi.on_wait = [w]
                e.sync_info = esi
                if ins.debug is not None:
                    e.debug = ins.debug
                carriers.append(e)
            si.on_wait = keep
            ins.sync_info = si
            for k, e in enumerate(carriers):
                insts.insert(idx + k, e)
            idx += len(carriers) + 1


def build_nc(variant="full"):
    import concourse.bass as bass
    import concourse.mybir as mybir
    import concourse.tile as tile

    BF16, F32 = mybir.dt.bfloat16, mybir.dt.float32
    AF = mybir.ActivationFunctionType
    ALU = mybir.AluOpType

    do_proj = variant in ("full", "nofc", "attn")
    do_attn = variant in ("full", "nofc", "attn")
    do_a2a = variant in ("full", "nofc")
    do_fc = variant in ("full",)
    do_x = variant != "empty"

    nc = bass.Bass()
    x_ext = nc.declare_dram_parameter("x", [N, D], F32, isOutput=False)
    wqk_ext = nc.declare_dram_parameter("wqk", [P, HPC * NC_CC, P], BF16, isOutput=False)
    wv_ext = nc.declare_dram_parameter("wv", [P, NC_CC, HPC * HD], BF16, isOutput=False)
    wfc_ext = nc.declare_dram_parameter("wfc", [P, 2 * NC_CC, D], BF16, isOutput=False)
    maskb_ext = nc.declare_dram_parameter("maskb", [P, NC_KC], F32, isOutput=False)
    bfc_ext = nc.declare_dram_parameter("bfc", [P, D], F32, isOutput=False)
    out_ext = nc.declare_dram_parameter("out", [512, D], F32, isOutput=True)

    with tile.TileContext(nc) as tc:
        with (
            tc.tile_pool(name="persist", bufs=1) as persist,
            tc.tile_pool(name="stage", bufs=2) as stage,
            tc.tile_pool(name="pt", bufs=2) as ptp,
            tc.tile_pool(name="ctx", bufs=2) as ctxp,
            tc.tile_pool(name="outp", bufs=2) as outp,
            tc.tile_pool(name="ps", bufs=2, space="PSUM") as ps,
            tc.tile_pool(name="ps_ctx", bufs=4, space="PSUM") as ps_ctx,
            tc.tile_pool(name="dram", bufs=1, space="DRAM") as dram,
        ):
            # ---- persistent SBUF tensors
            xT = persist.tile([P, NC_CC, N], BF16)  # x^T  [c, i]
            wqk = persist.tile([P, HPC * NC_CC, P], BF16)
            wv = persist.tile([P, NC_CC, HPC * HD], BF16)
            wfc = persist.tile([P, 2 * NC_CC, D], BF16)
            maskb = persist.tile([P, NC_KC], F32)
            bfc = persist.tile([P, D], F32)
            qT = [persist.tile([HD, N], BF16, name=f"qT{j}", tag=f"qT{j}")
                  for j in range(HPC)]
            kT = [persist.tile([HD, N], BF16, name=f"kT{j}", tag=f"kT{j}")
                  for j in range(HPC)]
            vv = persist.tile([P, NC_KC, HPC, HD + 1], BF16)  # V + ones col
            rec = persist.tile([1, N], F32)
            rbc = persist.tile([HD, N], F32)
            fcin = persist.tile([P, 2 * NC_CC, 512], BF16)

            nc.sync.dma_start(wqk[:], wqk_ext[:])
            nc.sync.dma_start(wv[:], wv_ext[:])
            nc.sync.dma_start(wfc[:], wfc_ext[:])
            nc.sync.dma_start(maskb[:], maskb_ext[:])
            nc.sync.dma_start(bfc[:], bfc_ext[:])
            nc.vector.memset(vv[:, :, :, HD:HD + 1], 1.0)

            # ---- DRAM internals
            xb_dram = dram.tile([N, D], BF16)
            a2a_in = dram.tile([NCORES, HPC * HD, 512], BF16, name="a2ai")
            a2a_out = dram.tile([NCORES, HPC * HD, 512], BF16, name="a2ao")
            rb_dram = dram.tile([HD, N], F32)

            # ---- phase 1: load x, cast bf16, transpose on PE
            if do_x:
                from concourse.masks import make_identity
                ident = persist.tile([P, P], BF16, name="ident", tag="ident")
                make_identity(nc, ident[:])
            for ic in range(NC_KC if do_x else 0):
                xf = stage.tile([P, D], F32, tag="xf")
                nc.sync.dma_start(xf[:], x_ext[ic * P:(ic + 1) * P, :])
                xb = stage.tile([P, D], BF16, tag="xb")
                nc.vector.tensor_copy(xb[:], xf[:])
                for cc in range(NC_CC):
                    ptr = ps.tile([P, P], BF16, tag="ps", name="ptr")
                    nc.tensor.transpose(ptr[:], xb[:, cc * P:(cc + 1) * P], ident[:])
                    nc.vector.tensor_copy(xT[:, cc, ic * P:(ic + 1) * P], ptr[:])

            # ---- phase 2: q/k projections (pair-stacked, pre-scaled q)
            for j in range(HPC if do_proj else 0):
                for i4 in range(NC_I4):
                    pqk = ps.tile([P, 512], mybir.dt.float32, tag="ps", name="pqk")
                    for cc in range(NC_CC):
                        nc.tensor.matmul(
                            pqk[:],
                            lhsT=wqk[:, j * NC_CC + cc, :],
                            rhs=xT[:, cc, i4 * 512:(i4 + 1) * 512],
                            start=(cc == 0),
                            stop=(cc == NC_CC - 1),
                        )
                    sl = slice(i4 * 512, (i4 + 1) * 512)
                    nc.vector.tensor_copy(qT[j][:, sl], pqk[0:HD, :])
                    nc.vector.tensor_copy(kT[j][:, sl], pqk[HD:P, :])

            # ---- phase 3: v projection (natural layout)
            for kc in range(NC_KC if do_proj else 0):
                pv = ps.tile([P, 512], mybir.dt.float32, tag="ps", name="pv")
                for cc in range(NC_CC):
                    nc.tensor.matmul(
                        pv[:, 0:HPC * HD],
                        lhsT=xT[:, cc, kc * P:(kc + 1) * P],
                        rhs=wv[:, cc, :],
                        start=(cc == 0),
                        stop=(cc == NC_CC - 1),
                    )
                nc.vector.tensor_copy(
                    vv[:, kc, :, 0:HD],
                    pv[:, 0:HPC * HD].rearrange("p (j d) -> p j d", j=HPC),
                )

            # ---- phase 4: attention per head (1024-wide softmax tiles)
            for j in range(HPC if do_attn else 0):
                pctx = [ps_ctx.tile([HD + 1, 512], mybir.dt.float32,
                                    tag="pctx", name=f"pctx{i4}")
                        for i4 in range(NC_I4)]
                for kc in range(NC_KC):
                    for i4 in range(NC_I4):
                        pss = ps.tile([P, 512], mybir.dt.float32, tag="ps",
                                      name="pss")
                        nc.tensor.matmul(
                            pss[:],
                            lhsT=kT[j][:, kc * P:(kc + 1) * P],
                            rhs=qT[j][:, i4 * 512:(i4 + 1) * 512],
                            start=True,
                            stop=True,
                        )
                        pT = ptp.tile([P, 512], BF16, tag="pT")
                        nc.scalar.activation(
                            pT[:], pss[:], AF.Exp,
                            bias=maskb[:, kc:kc + 1], scale=1.0,
                        )
                        nc.tensor.matmul(
                            pctx[i4][:],
                            lhsT=vv[:, kc, j, :],
                            rhs=pT[:],
                            start=(kc == 0),
                            stop=(kc == NC_KC - 1),
                        )
                # denominators -> reciprocal -> DRAM-doubling broadcast
                for i4 in range(NC_I4):
                    nc.vector.reciprocal(
                        rec[0:1, i4 * 512:(i4 + 1) * 512],
                        pctx[i4][HD:HD + 1, :],
                    )
                nc.sync.dma_start(rb_dram[0:1, :], rec[:])
                n = 1
                while n < HD:
                    m = min(n, HD - n)
                    nc.sync.dma_start(rb_dram[n:n + m, :], rb_dram[0:m, :])
                    n += m
                nc.sync.dma_start(rbc[:], rb_dram[:])
                # normalize into one per-head tile, ship to A2A input
                cst = ctxp.tile([HD, NC_I4, 512], BF16, tag="cst")
                for i4 in range(NC_I4):
                    nc.vector.tensor_tensor(
                        cst[:, i4, :],
                        pctx[i4][0:HD, :],
                        rbc[:, i4 * 512:(i4 + 1) * 512],
                        ALU.mult,
                    )
                for i4 in range(NC_I4):
                    nc.sync.dma_start(
                        a2a_in[i4, j * HD:(j + 1) * HD, :], cst[:, i4, :])
                    nc.sync.dma_start(
                        a2a_in[i4 + 4, j * HD:(j + 1) * HD, :], cst[:, i4, :])
            if do_a2a:
                nc.gpsimd.collective_compute(
                    "AllToAll",
                    mybir.AluOpType.bypass,
                    replica_groups=[list(range(NCORES))],
                    ins=[a2a_in.opt()],
                    outs=[a2a_out.opt()],
                )

            # ---- phase 6: FC over 1536-row gathered ctx^T, add bias
            if not do_fc:
                ob0 = outp.tile([P, D], F32, tag="ob", name="ob0")
                nc.vector.memset(ob0[:], 0.0)
                for i4 in range(NC_I4):
                    nc.sync.dma_start(out_ext[i4 * P:(i4 + 1) * P, :], ob0[:])
            if do_fc:
                a2a_flat = a2a_out[:].rearrange("r d q -> (r d) q")
                for cc in range(2 * NC_CC):
                    nc.gpsimd.dma_start(
                        fcin[:, cc, :], a2a_flat[cc * P:(cc + 1) * P, :])
            for i4 in range(NC_I4 if do_fc else 0):
                pfa = ps.tile([P, 512], mybir.dt.float32, tag="ps", name="pfa")
                pfb = ps.tile([P, 512], mybir.dt.float32, tag="ps", name="pfb")
                for cc in range(2 * NC_CC):
                    lhsT = fcin[:, cc, i4 * P:(i4 + 1) * P]
                    nc.tensor.matmul(
                        pfa[:], lhsT=lhsT, rhs=wfc[:, cc, 0:512],
                        start=(cc == 0), stop=(cc == 2 * NC_CC - 1))
                    nc.tensor.matmul(
                        pfb[:, 0:D - 512], lhsT=lhsT, rhs=wfc[:, cc, 512:D],
                        start=(cc == 0), stop=(cc == 2 * NC_CC - 1))
                ob = outp.tile([P, D], F32, tag="ob", name="ob")
                nc.vector.tensor_tensor(ob[:, 0:512], pfa[:], bfc[:, 0:512], ALU.add)
                nc.vector.tensor_tensor(ob[:, 512:D], pfb[:, 0:D - 512],
                                        bfc[:, 512:D], ALU.add)
                nc.sync.dma_start(out_ext[i4 * P:(i4 + 1) * P, :], ob[:])

    _fix_multi_waits(nc)
    return nc


def _prep_in_maps(inputs, padding_mask, w_qkv, w_fc, b_fc):
    in_maps = []
    for c in range(NCORES):
        g, q4 = c // 4, c % 4
        x = np.ascontiguousarray(inputs[g], dtype=np.float32)

        # wqk[p, j*6+cc, m]: m<64 -> scaled WqT, else WkT
        wqk = np.empty((P, HPC * NC_CC, P), dtype=np.float32)
        for jj in range(HPC):
            h = 3 * q4 + jj
            wq = w_qkv[h * HD:(h + 1) * HD, :] * SCALE        # [64, 768]
            wk = w_qkv[D + h * HD:D + (h + 1) * HD, :]        # [64, 768]
            for cc in range(NC_CC):
                wqk[:, jj * NC_CC + cc, 0:HD] = wq[:, cc * P:(cc + 1) * P].T
                wqk[:, jj * NC_CC + cc, HD:P] = wk[:, cc * P:(cc + 1) * P].T

        wv = np.empty((P, NC_CC, HPC * HD), dtype=np.float32)
        for jj in range(HPC):
            h = 3 * q4 + jj
            wvh = w_qkv[2 * D + h * HD:2 * D + (h + 1) * HD, :]  # [64, 768]
            for cc in range(NC_CC):
                wv[:, cc, jj * HD:(jj + 1) * HD] = wvh[:, cc * P:(cc + 1) * P].T

        # wfc[p, cc12, e] over 1536 gathered rows; zero rows for foreign ranks
        wfc_rows = np.zeros((NCORES * HPC * HD, D), dtype=np.float32)
        for s in range(NCORES):
            if s // 4 != g:
                continue
            for jj in range(HPC):
                h = 3 * (s % 4) + jj
                wfc_rows[s * HPC * HD + jj * HD:
                         s * HPC * HD + (jj + 1) * HD, :] = \
                    w_fc[:, h * HD:(h + 1) * HD].T
        wfc = wfc_rows.reshape(2 * NC_CC, P, D).transpose(1, 0, 2)

        maskb = (MASK_VAL * (padding_mask[g] > 0)).astype(np.float32)
        maskb = maskb.reshape(NC_KC, P).T.copy()  # [p, kc]

        bfc = np.tile(np.asarray(b_fc, dtype=np.float32)[None, :], (P, 1))

        in_maps.append({
            "x": x,
            "wqk": np.ascontiguousarray(wqk).astype(_BF16),
            "wv": np.ascontiguousarray(wv).astype(_BF16),
            "wfc": np.ascontiguousarray(wfc).astype(_BF16),
            "maskb": maskb,
            "bfc": bfc,
        })
    return in_maps


_CACHED_NC = None


def get_nc():
    global _CACHED_NC
    if _CACHED_NC is None:
        _CACHED_NC = build_nc()
    return _CACHED_NC


def kernel(inputs, padding_mask, w_qkv, w_fc, b_fc):
    inputs = np.asarray(inputs)
    padding_mask = np.asarray(padding_mask)
    w_qkv = np.asarray(w_qkv, dtype=np.float32)
    w_fc = np.asarray(w_fc, dtype=np.float32)
    b_fc = np.asarray(b_fc, dtype=np.float32)

    from concourse.bass_utils import run_bass_kernel_spmd

    nc = get_nc()
    in_maps = _prep_in_maps(inputs, padding_mask, w_qkv, w_fc, b_fc)
    res = run_bass_kernel_spmd(nc, in_maps, list(range(NCORES)))
    out = np.empty((B, N, D), dtype=np.float32)
    for c in range(NCORES):
        out[c // 4, (c % 4) * 512:(c % 4 + 1) * 512, :] = res.results[c]["out"]
    return out


# revision 16
# speedup vs baseline: 8.5652x; 8.5652x over previous
"""Distributed Trainium2 kernel for nn_Attention (B=2, N=2048, D=768, H=12).

Sharding: core c handles batch c//4 and head-triple c%4 (3 heads) for the
attention; the FC output projection is query-split (core c computes rows
(c%4)*512 .. +512 of its batch). The context tensor moves between the two
shardings with per-head 8-core AllToAlls of ctx^T column blocks; the FC
contracts over all 8 ranks' head-dims with rows of w_fc^T zeroed for the
other batch's ranks, which keeps the SPMD program identical on every core.

Compute dtype: bf16 matmul operands, fp32 PSUM accumulation and softmax.
The padding mask is folded into the exp bias (-1e7 per masked key), the
1/sqrt(hd) scale into the pre-transposed q weights, and the softmax
denominator rides the PV matmul as a 65th ones-column of V.
"""

import sys
import numpy as np

sys.path.insert(0, "/opt/trn_rl_repo")

import ml_dtypes

B, N, D, H, HD = 2, 2048, 768, 12, 64
P = 128
NCORES = 8
HPC = 3  # heads per core
NC_I4 = N // 512
NC_KC = N // P
NC_CC = D // P
SCALE = HD ** (-0.5)
MASK_VAL = -10000000.0

_BF16 = ml_dtypes.bfloat16


def _fix_multi_waits(nc):
    """walrus in this container accepts only ONE semaphore wait per
    instruction; hoist extra waits onto EventSemaphore carriers inserted
    immediately before, on the same engine (program order preserved)."""
    import bass_rust

    for b in nc.main_func.blocks:
        insts = b.instructions
        idx = 0
        while idx < len(insts):
            ins = insts[idx]
            si = ins.sync_info
            if si is None or len(si.on_wait) <= 1:
                idx += 1
                continue
            waits = list(si.on_wait)
            excess, keep = waits[:-1], waits[-1:]
            carriers = []
            for k, w in enumerate(excess):
                e = bass_rust.InstEventSemaphore(
                    name=f"{ins.name}_waitsplit_{k}", ins=[], outs=[]
                )
                e.engine = ins.engine
                esi = e.sync_info
                if esi is None:
                    esi = bass_rust.SyncInfo(on_wait=[], on_update=[])
                esi.on_wait = [w]
                e.sync_info = esi
                if ins.debug is not None:
                    e.debug = ins.debug
                carriers.append(e)
            si.on_wait = keep
            ins.sync_info = si
            for k, e in enumerate(carriers):
                insts.insert(idx + k, e)
            idx += len(carriers) + 1


def build_nc(variant="full"):
    import concourse.bass as bass
    import concourse.mybir as mybir
    import concourse.tile as tile

    BF16, F32 = mybir.dt.bfloat16, mybir.dt.float32
    AF = mybir.ActivationFunctionType
    ALU = mybir.AluOpType

    do_proj = variant in ("full", "nofc", "attn")
    do_attn = variant in ("full", "nofc", "attn")
    do_a2a = variant in ("full", "nofc")
    do_fc = variant in ("full",)
    do_x = variant != "empty"

    nc = bass.Bass()
    x_ext = nc.declare_dram_parameter("x", [N, D], BF16, isOutput=False)
    wqk_ext = nc.declare_dram_parameter("wqk", [P, HPC * NC_CC, P], BF16, isOutput=False)
    wv_ext = nc.declare_dram_parameter("wv", [P, NC_CC, HPC * HD], BF16, isOutput=False)
    wfc_ext = nc.declare_dram_parameter("wfc", [P, 2 * NC_CC, D], BF16, isOutput=False)
    maskb_ext = nc.declare_dram_parameter("maskb", [P, NC_KC], F32, isOutput=False)
    bfc_ext = nc.declare_dram_parameter("bfc", [P, D], F32, isOutput=False)
    out_ext = nc.declare_dram_parameter("out", [512, D], F32, isOutput=True)

    with tile.TileContext(nc) as tc:
        with (
            tc.tile_pool(name="persist", bufs=1) as persist,
            tc.tile_pool(name="stage", bufs=2) as stage,
            tc.tile_pool(name="pt", bufs=3) as ptp,
            tc.tile_pool(name="ctx", bufs=2) as ctxp,
            tc.tile_pool(name="outp", bufs=2) as outp,
            tc.tile_pool(name="ps", bufs=2, space="PSUM") as ps,
            tc.tile_pool(name="ps_ctx", bufs=4, space="PSUM") as ps_ctx,
            tc.tile_pool(name="dram", bufs=1, space="DRAM") as dram,
        ):
            # ---- persistent SBUF tensors
            xTq = [persist.tile([P, NC_CC, 512], BF16, name=f"xTq{q}", tag=f"xTq{q}")
                   for q in range(NC_I4)]  # x^T  [c, i] by seq quarter
            wqk = persist.tile([P, HPC * NC_CC, P], BF16)
            wv = persist.tile([P, NC_CC, HPC * HD], BF16)
            wfc = persist.tile([P, 2 * NC_CC, D], BF16)
            maskb = persist.tile([P, NC_KC], F32)
            bfc = persist.tile([P, D], F32)
            qT = [persist.tile([HD, N], BF16, name=f"qT{j}", tag=f"qT{j}")
                  for j in range(HPC)]
            kT = [persist.tile([HD, N], BF16, name=f"kT{j}", tag=f"kT{j}")
                  for j in range(HPC)]
            vv = persist.tile([P, NC_KC, HPC, HD + 1], BF16)  # V + ones col
            rec = persist.tile([1, N], F32)
            rbc = persist.tile([HD, N], F32)
            fcin = persist.tile([P, 2 * NC_CC, 512], BF16)

            nc.sync.dma_start(wqk[:], wqk_ext[:])
            nc.sync.dma_start(wv[:], wv_ext[:])
            nc.sync.dma_start(wfc[:], wfc_ext[:])
            nc.sync.dma_start(maskb[:], maskb_ext[:])
            nc.sync.dma_start(bfc[:], bfc_ext[:])
            nc.vector.memset(vv[:, :, :, HD:HD + 1], 1.0)

            # ---- DRAM internals
            a2a_in = [dram.tile([NCORES, HD, 512], BF16, name=f"a2ai{j}")
                      for j in range(HPC)]
            a2a_out = [dram.tile([NCORES, HD, 512], BF16, name=f"a2ao{j}")
                       for j in range(HPC)]
            rb_dram = dram.tile([HD, N], F32)

            # ---- phase 1: XBAR-transpose x (already bf16) from DRAM,
            #      split by sequence quarter so projections pipeline
            for q in range(NC_I4 if do_x else 0):
                for cc in range(NC_CC):
                    nc.sync.dma_start_transpose(
                        xTq[q][:, cc, :],
                        x_ext[q * 512:(q + 1) * 512, cc * P:(cc + 1) * P])

            # ---- phase 2: q/k projections (pair-stacked, pre-scaled q)
            for j in range(HPC if do_proj else 0):
                for i4 in range(NC_I4):
                    pqk = ps.tile([P, 512], mybir.dt.float32, tag="ps", name="pqk")
                    for cc in range(NC_CC):
                        nc.tensor.matmul(
                            pqk[:],
                            lhsT=wqk[:, j * NC_CC + cc, :],
                            rhs=xTq[i4][:, cc, :],
                            start=(cc == 0),
                            stop=(cc == NC_CC - 1),
                        )
                    sl = slice(i4 * 512, (i4 + 1) * 512)
                    nc.vector.tensor_copy(qT[j][:, sl], pqk[0:HD, :])
                    nc.vector.tensor_copy(kT[j][:, sl], pqk[HD:P, :])

            # ---- phase 3: v projection (natural layout)
            for kc in range(NC_KC if do_proj else 0):
                pv = ps.tile([P, 512], mybir.dt.float32, tag="ps", name="pv")
                for cc in range(NC_CC):
                    nc.tensor.matmul(
                        pv[:, 0:HPC * HD],
                        lhsT=xTq[kc // 4][:, cc, (kc % 4) * P:(kc % 4 + 1) * P],
                        rhs=wv[:, cc, :],
                        start=(cc == 0),
                        stop=(cc == NC_CC - 1),
                    )
                nc.vector.tensor_copy(
                    vv[:, kc, :, 0:HD],
                    pv[:, 0:HPC * HD].rearrange("p (j d) -> p j d", j=HPC),
                )

            # ---- phase 4: attention per head (1024-wide softmax tiles)
            for j in range(HPC if do_attn else 0):
                pctx = [ps_ctx.tile([HD + 1, 512], mybir.dt.float32,
                                    tag="pctx", name=f"pctx{i4}")
                        for i4 in range(NC_I4)]
                for kc in range(NC_KC):
                    for ih in range(2):
                        pss = ps.tile([P, 1024], mybir.dt.float32, tag="ps",
                                      name="pss")
                        for i2 in range(2):
                            i4 = ih * 2 + i2
                            nc.tensor.matmul(
                                pss[:, i2 * 512:(i2 + 1) * 512],
                                lhsT=kT[j][:, kc * P:(kc + 1) * P],
                                rhs=qT[j][:, i4 * 512:(i4 + 1) * 512],
                                start=True,
                                stop=True,
                            )
                        pT = ptp.tile([P, 1024], BF16, tag="pT")
                        nc.scalar.activation(
                            pT[:], pss[:], AF.Exp,
                            bias=maskb[:, kc:kc + 1], scale=1.0,
                        )
                        for i2 in range(2):
                            i4 = ih * 2 + i2
                            nc.tensor.matmul(
                                pctx[i4][:],
                                lhsT=vv[:, kc, j, :],
                                rhs=pT[:, i2 * 512:(i2 + 1) * 512],
                                start=(kc == 0),
                                stop=(kc == NC_KC - 1),
                            )
                # quick-release PSUM: copy unnormalized ctx + denom to SBUF
                ub = ctxp.tile([HD + 1, NC_I4, 512], mybir.dt.float32, tag="ub",
                               name="ub")
                for i4 in range(NC_I4):
                    nc.vector.tensor_copy(ub[:, i4, :], pctx[i4][:])
                # reciprocal -> DRAM-doubling broadcast
                nc.vector.reciprocal(rec[0:1, :],
                                     ub[HD:HD + 1, :, :].rearrange("e q n -> e (q n)"))
                nc.sync.dma_start(rb_dram[0:1, :], rec[:])
                n = 1
                while n < HD:
                    m = min(n, HD - n)
                    nc.sync.dma_start(rb_dram[n:n + m, :], rb_dram[0:m, :])
                    n += m
                nc.sync.dma_start(rbc[:], rb_dram[:])
                # normalize into one per-head tile, ship to A2A input
                cst = ctxp.tile([HD, NC_I4, 512], BF16, tag="cst")
                nc.vector.tensor_tensor(
                    cst[:].rearrange("d q n -> d (q n)"),
                    ub[0:HD, :, :].rearrange("d q n -> d (q n)"),
                    rbc[:],
                    ALU.mult,
                )
                for i4 in range(NC_I4):
                    nc.sync.dma_start(
                        a2a_in[j][i4, :, :], cst[:, i4, :])
                    nc.sync.dma_start(
                        a2a_in[j][i4 + 4, :, :], cst[:, i4, :])
                if do_a2a:
                    nc.gpsimd.collective_compute(
                        "AllToAll",
                        mybir.AluOpType.bypass,
                        replica_groups=[list(range(NCORES))],
                        ins=[a2a_in[j].opt()],
                        outs=[a2a_out[j].opt()],
                    )

            # ---- phase 6: FC over 1536-row gathered ctx^T, add bias
            if not do_fc:
                ob0 = outp.tile([P, D], F32, tag="ob", name="ob0")
                nc.vector.memset(ob0[:], 0.0)
                for i4 in range(NC_I4):
                    nc.sync.dma_start(out_ext[i4 * P:(i4 + 1) * P, :], ob0[:])
            # gather: global contraction row r = s*192 + j*64 + dd comes from
            # a2a_out[j][s, dd, :]; per 64-row granule.
            for gr in range(2 * NC_CC * 2 if do_fc else 0):
                r = gr * HD
                s_, j_ = r // (HPC * HD), (r % (HPC * HD)) // HD
                cc, half = gr // 2, gr % 2
                nc.gpsimd.dma_start(
                    fcin[half * HD:(half + 1) * HD, cc, :],
                    a2a_out[j_][s_, :, :])
            cc_order = sorted(range(2 * NC_CC),
                              key=lambda cc: max((cc * P + half * HD) % (HPC * HD) // HD
                                                 for half in range(2)))
            for i4 in range(NC_I4 if do_fc else 0):
                pfa = ps.tile([P, 512], mybir.dt.float32, tag="ps", name="pfa")
                pfb = ps.tile([P, 512], mybir.dt.float32, tag="ps", name="pfb")
                for ci, cc in enumerate(cc_order):
                    lhsT = fcin[:, cc, i4 * P:(i4 + 1) * P]
                    nc.tensor.matmul(
                        pfa[:], lhsT=lhsT, rhs=wfc[:, cc, 0:512],
                        start=(ci == 0), stop=(ci == 2 * NC_CC - 1))
                    nc.tensor.matmul(
                        pfb[:, 0:D - 512], lhsT=lhsT, rhs=wfc[:, cc, 512:D],
                        start=(ci == 0), stop=(ci == 2 * NC_CC - 1))
                ob = outp.tile([P, D], F32, tag="ob", name="ob")
                nc.vector.tensor_tensor(ob[:, 0:512], pfa[:], bfc[:, 0:512], ALU.add)
                nc.vector.tensor_tensor(ob[:, 512:D], pfb[:, 0:D - 512],
                                        bfc[:, 512:D], ALU.add)
                nc.sync.dma_start(out_ext[i4 * P:(i4 + 1) * P, :], ob[:])

    _fix_multi_waits(nc)
    return nc


def _prep_in_maps(inputs, padding_mask, w_qkv, w_fc, b_fc):
    in_maps = []
    for c in range(NCORES):
        g, q4 = c // 4, c % 4
        x = np.ascontiguousarray(inputs[g], dtype=np.float32).astype(_BF16)

        # wqk[p, j*6+cc, m]: m<64 -> scaled WqT, else WkT
        wqk = np.empty((P, HPC * NC_CC, P), dtype=np.float32)
        for jj in range(HPC):
            h = 3 * q4 + jj
            wq = w_qkv[h * HD:(h + 1) * HD, :] * SCALE        # [64, 768]
            wk = w_qkv[D + h * HD:D + (h + 1) * HD, :]        # [64, 768]
            for cc in range(NC_CC):
                wqk[:, jj * NC_CC + cc, 0:HD] = wq[:, cc * P:(cc + 1) * P].T
                wqk[:, jj * NC_CC + cc, HD:P] = wk[:, cc * P:(cc + 1) * P].T

        wv = np.empty((P, NC_CC, HPC * HD), dtype=np.float32)
        for jj in range(HPC):
            h = 3 * q4 + jj
            wvh = w_qkv[2 * D + h * HD:2 * D + (h + 1) * HD, :]  # [64, 768]
            for cc in range(NC_CC):
                wv[:, cc, jj * HD:(jj + 1) * HD] = wvh[:, cc * P:(cc + 1) * P].T

        # wfc[p, cc12, e] over 1536 gathered rows; zero rows for foreign ranks
        wfc_rows = np.zeros((NCORES * HPC * HD, D), dtype=np.float32)
        for s in range(NCORES):
            if s // 4 != g:
                continue
            for jj in range(HPC):
                h = 3 * (s % 4) + jj
                wfc_rows[s * HPC * HD + jj * HD:
                         s * HPC * HD + (jj + 1) * HD, :] = \
                    w_fc[:, h * HD:(h + 1) * HD].T
        wfc = wfc_rows.reshape(2 * NC_CC, P, D).transpose(1, 0, 2)

        maskb = (MASK_VAL * (padding_mask[g] > 0)).astype(np.float32)
        maskb = maskb.reshape(NC_KC, P).T.copy()  # [p, kc]

        bfc = np.tile(np.asarray(b_fc, dtype=np.float32)[None, :], (P, 1))

        in_maps.append({
            "x": x,
            "wqk": np.ascontiguousarray(wqk).astype(_BF16),
            "wv": np.ascontiguousarray(wv).astype(_BF16),
            "wfc": np.ascontiguousarray(wfc).astype(_BF16),
            "maskb": maskb,
            "bfc": bfc,
        })
    return in_maps


_CACHED_NC = None


def get_nc():
    global _CACHED_NC
    if _CACHED_NC is None:
        _CACHED_NC = build_nc()
    return _CACHED_NC


def kernel(inputs, padding_mask, w_qkv, w_fc, b_fc):
    inputs = np.asarray(inputs)
    padding_mask = np.asarray(padding_mask)
    w_qkv = np.asarray(w_qkv, dtype=np.float32)
    w_fc = np.asarray(w_fc, dtype=np.float32)
    b_fc = np.asarray(b_fc, dtype=np.float32)

    from concourse.bass_utils import run_bass_kernel_spmd

    nc = get_nc()
    in_maps = _prep_in_maps(inputs, padding_mask, w_qkv, w_fc, b_fc)
    res = run_bass_kernel_spmd(nc, in_maps, list(range(NCORES)))
    out = np.empty((B, N, D), dtype=np.float32)
    for c in range(NCORES):
        out[c // 4, (c % 4) * 512:(c % 4 + 1) * 512, :] = res.results[c]["out"]
    return out


# revision 19
# speedup vs baseline: 19.1615x; 2.2371x over previous
"""Distributed Trainium2 kernel for nn_Attention (B=2, N=2048, D=768, H=12).

Sharding: core c handles batch c//4 and head-triple c%4 (3 heads) for the
attention; the FC output projection is query-split (core c computes rows
(c%4)*512 .. +512 of its batch). The context tensor moves between the two
shardings with per-head 8-core AllToAlls of ctx^T column blocks; the FC
contracts over all 8 ranks' head-dims with rows of w_fc^T zeroed for the
other batch's ranks, which keeps the SPMD program identical on every core.

Compute dtype: bf16 matmul operands, fp32 PSUM accumulation and softmax.
The padding mask is folded into the exp bias (-1e7 per masked key), the
1/sqrt(hd) scale into the pre-transposed q weights, and the softmax
denominator rides the PV matmul as a 65th ones-column of V.
"""

import sys
import numpy as np

sys.path.insert(0, "/opt/trn_rl_repo")

import ml_dtypes

B, N, D, H, HD = 2, 2048, 768, 12, 64
P = 128
NCORES = 8
HPC = 3  # heads per core
NC_I4 = N // 512
NC_KC = N // P
NC_CC = D // P
SCALE = HD ** (-0.5)
MASK_VAL = -10000000.0

_BF16 = ml_dtypes.bfloat16


def _fix_multi_waits(nc):
    """walrus in this container accepts only ONE semaphore wait per
    instruction; hoist extra waits onto EventSemaphore carriers inserted
    immediately before, on the same engine (program order preserved)."""
    import bass_rust

    for b in nc.main_func.blocks:
        insts = b.instructions
        idx = 0
        while idx < len(insts):
            ins = insts[idx]
            si = ins.sync_info
            if si is None or len(si.on_wait) <= 1:
                idx += 1
                continue
            waits = list(si.on_wait)
            excess, keep = waits[:-1], waits[-1:]
            carriers = []
            for k, w in enumerate(excess):
                e = bass_rust.InstEventSemaphore(
                    name=f"{ins.name}_waitsplit_{k}", ins=[], outs=[]
                )
                e.engine = ins.engine
                esi = e.sync_info
                if esi is None:
                    esi = bass_rust.SyncInfo(on_wait=[], on_update=[])
                esi.on_wait = [w]
                e.sync_info = esi
                if ins.debug is not None:
                    e.debug = ins.debug
                carriers.append(e)
            si.on_wait = keep
            ins.sync_info = si
            for k, e in enumerate(carriers):
                insts.insert(idx + k, e)
            idx += len(carriers) + 1


def build_nc(variant="full"):
    import concourse.bass as bass
    import concourse.mybir as mybir
    import concourse.tile as tile

    BF16, F32 = mybir.dt.bfloat16, mybir.dt.float32
    AF = mybir.ActivationFunctionType
    ALU = mybir.AluOpType

    do_proj = variant in ("full", "nofc", "attn")
    do_attn = variant in ("full", "nofc", "attn")
    do_a2a = variant in ("full", "nofc")
    do_fc = variant in ("full",)
    do_x = variant != "empty"

    nc = bass.Bass()
    x_ext = nc.declare_dram_parameter("x", [N, D], BF16, isOutput=False)
    wqk_ext = nc.declare_dram_parameter("wqk", [P, HPC * NC_CC, P], BF16, isOutput=False)
    wv_ext = nc.declare_dram_parameter("wv", [P, NC_CC, HPC * HD], BF16, isOutput=False)
    wfc_ext = nc.declare_dram_parameter("wfc", [P, 2 * NC_CC, D], BF16, isOutput=False)
    maskb_ext = nc.declare_dram_parameter("maskb", [P, NC_KC], F32, isOutput=False)
    bfc_ext = nc.declare_dram_parameter("bfc", [P, D], F32, isOutput=False)
    out_ext = nc.declare_dram_parameter("out", [512, D], F32, isOutput=True)

    with tile.TileContext(nc) as tc:
        with (
            tc.tile_pool(name="persist", bufs=1) as persist,
            tc.tile_pool(name="stage", bufs=2) as stage,
            tc.tile_pool(name="pt", bufs=3) as ptp,
            tc.tile_pool(name="ctx", bufs=2) as ctxp,
            tc.tile_pool(name="outp", bufs=2) as outp,
            tc.tile_pool(name="ps", bufs=2, space="PSUM") as ps,
            tc.tile_pool(name="ps_ctx", bufs=4, space="PSUM") as ps_ctx,
            tc.tile_pool(name="dram", bufs=1, space="DRAM") as dram,
        ):
            # ---- persistent SBUF tensors
            xTq = [persist.tile([P, NC_CC, 512], BF16, name=f"xTq{q}", tag=f"xTq{q}")
                   for q in range(NC_I4)]  # x^T  [c, i] by seq quarter
            wqk = persist.tile([P, HPC * NC_CC, P], BF16)
            wv = persist.tile([P, NC_CC, HPC * HD], BF16)
            wfc = persist.tile([P, 2 * NC_CC, D], BF16)
            maskb = persist.tile([P, NC_KC], F32)
            bfc = persist.tile([P, D], F32)
            qT = [persist.tile([HD, N], BF16, name=f"qT{j}", tag=f"qT{j}")
                  for j in range(HPC)]
            kT = [persist.tile([HD, N], BF16, name=f"kT{j}", tag=f"kT{j}")
                  for j in range(HPC)]
            vv = persist.tile([P, NC_KC, HPC, HD + 1], BF16)  # V + ones col
            rec = persist.tile([1, N], F32)
            rbc = persist.tile([HD, N], F32)
            fcin = persist.tile([P, 2 * NC_CC, 512], BF16)

            nc.sync.dma_start(wqk[:], wqk_ext[:])
            nc.sync.dma_start(wv[:], wv_ext[:])
            nc.sync.dma_start(wfc[:], wfc_ext[:])
            nc.sync.dma_start(maskb[:], maskb_ext[:])
            nc.sync.dma_start(bfc[:], bfc_ext[:])
            nc.vector.memset(vv[:, :, :, HD:HD + 1], 1.0)

            # ---- DRAM internals
            a2a_in = [dram.tile([NCORES, HD, 512], BF16, name=f"a2ai{j}")
                      for j in range(HPC)]
            a2a_out = [dram.tile([NCORES, HD, 512], BF16, name=f"a2ao{j}")
                       for j in range(HPC)]
            rb_dram = dram.tile([HD, N], F32)

            # ---- phase 1: XBAR-transpose x (already bf16) from DRAM,
            #      split by sequence quarter so projections pipeline
            for q in range(NC_I4 if do_x else 0):
                for cc in range(NC_CC):
                    nc.sync.dma_start_transpose(
                        xTq[q][:, cc, :],
                        x_ext[q * 512:(q + 1) * 512, cc * P:(cc + 1) * P])

            # ---- phase 2: q/k projections (pair-stacked, pre-scaled q)
            for j in range(HPC if do_proj else 0):
                for i4 in range(NC_I4):
                    pqk = ps.tile([P, 512], mybir.dt.float32, tag="ps", name="pqk")
                    for cc in range(NC_CC):
                        nc.tensor.matmul(
                            pqk[:],
                            lhsT=wqk[:, j * NC_CC + cc, :],
                            rhs=xTq[i4][:, cc, :],
                            start=(cc == 0),
                            stop=(cc == NC_CC - 1),
                        )
                    sl = slice(i4 * 512, (i4 + 1) * 512)
                    nc.vector.tensor_copy(qT[j][:, sl], pqk[0:HD, :])
                    nc.vector.tensor_copy(kT[j][:, sl], pqk[HD:P, :])

            # ---- phase 3: v projection (natural layout)
            for kc in range(NC_KC if do_proj else 0):
                pv = ps.tile([P, 512], mybir.dt.float32, tag="ps", name="pv")
                for cc in range(NC_CC):
                    nc.tensor.matmul(
                        pv[:, 0:HPC * HD],
                        lhsT=xTq[kc // 4][:, cc, (kc % 4) * P:(kc % 4 + 1) * P],
                        rhs=wv[:, cc, :],
                        start=(cc == 0),
                        stop=(cc == NC_CC - 1),
                    )
                nc.vector.tensor_copy(
                    vv[:, kc, :, 0:HD],
                    pv[:, 0:HPC * HD].rearrange("p (j d) -> p j d", j=HPC),
                )

            # ---- phase 4: attention per head (1024-wide softmax tiles)
            for j in range(HPC if do_attn else 0):
                pctx = [ps_ctx.tile([HD + 1, 512], mybir.dt.float32,
                                    tag="pctx", name=f"pctx{i4}")
                        for i4 in range(NC_I4)]
                for kc in range(NC_KC):
                    for ih in range(2):
                        pss = ps.tile([P, 1024], mybir.dt.float32, tag="ps",
                                      name="pss")
                        for i2 in range(2):
                            i4 = ih * 2 + i2
                            nc.tensor.matmul(
                                pss[:, i2 * 512:(i2 + 1) * 512],
                                lhsT=kT[j][:, kc * P:(kc + 1) * P],
                                rhs=qT[j][:, i4 * 512:(i4 + 1) * 512],
                                start=True,
                                stop=True,
                            )
                        pT = ptp.tile([P, 1024], BF16, tag="pT")
                        nc.scalar.activation(
                            pT[:], pss[:], AF.Exp,
                            bias=maskb[:, kc:kc + 1], scale=1.0,
                        )
                        for i2 in range(2):
                            i4 = ih * 2 + i2
                            nc.tensor.matmul(
                                pctx[i4][:],
                                lhsT=vv[:, kc, j, :],
                                rhs=pT[:, i2 * 512:(i2 + 1) * 512],
                                start=(kc == 0),
                                stop=(kc == NC_KC - 1),
                            )
                # quick-release PSUM: copy unnormalized ctx + denom to SBUF
                ub = ctxp.tile([HD + 1, NC_I4, 512], mybir.dt.float32, tag="ub",
                               name="ub")
                for i4 in range(NC_I4):
                    nc.vector.tensor_copy(ub[:, i4, :], pctx[i4][:])
                # reciprocal -> DRAM-doubling broadcast
                nc.vector.reciprocal(rec[0:1, :],
                                     ub[HD:HD + 1, :, :].rearrange("e q n -> e (q n)"))
                nc.sync.dma_start(rb_dram[0:1, :], rec[:])
                n = 1
                while n < HD:
                    m = min(n, HD - n)
                    nc.sync.dma_start(rb_dram[n:n + m, :], rb_dram[0:m, :])
                    n += m
                nc.sync.dma_start(rbc[:], rb_dram[:])
                # normalize into one per-head tile, ship to A2A input
                cst = ctxp.tile([HD, NC_I4, 512], BF16, tag="cst")
                nc.vector.tensor_tensor(
                    cst[:].rearrange("d q n -> d (q n)"),
                    ub[0:HD, :, :].rearrange("d q n -> d (q n)"),
                    rbc[:],
                    ALU.mult,
                )
                for i4 in range(NC_I4):
                    nc.sync.dma_start(
                        a2a_in[j][i4, :, :], cst[:, i4, :])
                    nc.sync.dma_start(
                        a2a_in[j][i4 + 4, :, :], cst[:, i4, :])
                if do_a2a:
                    nc.gpsimd.collective_compute(
                        "AllToAll",
                        mybir.AluOpType.bypass,
                        replica_groups=[list(range(NCORES))],
                        ins=[a2a_in[j].opt()],
                        outs=[a2a_out[j].opt()],
                    )

            # ---- phase 6: FC over 1536-row gathered ctx^T, add bias
            if not do_fc:
                ob0 = outp.tile([P, D], F32, tag="ob", name="ob0")
                nc.vector.memset(ob0[:], 0.0)
                for i4 in range(NC_I4):
                    nc.sync.dma_start(out_ext[i4 * P:(i4 + 1) * P, :], ob0[:])
            # gather: global contraction row r = s*192 + j*64 + dd comes from
            # a2a_out[j][s, dd, :]; per 64-row granule.
            for gr in range(2 * NC_CC * 2 if do_fc else 0):
                r = gr * HD
                s_, j_ = r // (HPC * HD), (r % (HPC * HD)) // HD
                cc, half = gr // 2, gr % 2
                nc.gpsimd.dma_start(
                    fcin[half * HD:(half + 1) * HD, cc, :],
                    a2a_out[j_][s_, :, :])
            cc_order = sorted(range(2 * NC_CC),
                              key=lambda cc: max((cc * P + half * HD) % (HPC * HD) // HD
                                                 for half in range(2)))
            for i4 in range(NC_I4 if do_fc else 0):
                pfa = ps.tile([P, 512], mybir.dt.float32, tag="ps", name="pfa")
                pfb = ps.tile([P, 512], mybir.dt.float32, tag="ps", name="pfb")
                for ci, cc in enumerate(cc_order):
                    lhsT = fcin[:, cc, i4 * P:(i4 + 1) * P]
                    nc.tensor.matmul(
                        pfa[:], lhsT=lhsT, rhs=wfc[:, cc, 0:512],
                        start=(ci == 0), stop=(ci == 2 * NC_CC - 1))
                    nc.tensor.matmul(
                        pfb[:, 0:D - 512], lhsT=lhsT, rhs=wfc[:, cc, 512:D],
                        start=(ci == 0), stop=(ci == 2 * NC_CC - 1))
                ob = outp.tile([P, D], F32, tag="ob", name="ob")
                nc.vector.tensor_tensor(ob[:, 0:512], pfa[:], bfc[:, 0:512], ALU.add)
                nc.vector.tensor_tensor(ob[:, 512:D], pfb[:, 0:D - 512],
                                        bfc[:, 512:D], ALU.add)
                nc.sync.dma_start(out_ext[i4 * P:(i4 + 1) * P, :], ob[:])

    _fix_multi_waits(nc)
    return nc


def _prep_in_maps(inputs, padding_mask, w_qkv, w_fc, b_fc):
    in_maps = []
    for c in range(NCORES):
        g, q4 = c // 4, c % 4
        x = np.ascontiguousarray(inputs[g], dtype=np.float32).astype(_BF16)

        # wqk[p, j*6+cc, m]: m<64 -> scaled WqT, else WkT
        wqk = np.empty((P, HPC * NC_CC, P), dtype=np.float32)
        for jj in range(HPC):
            h = 3 * q4 + jj
            wq = w_qkv[h * HD:(h + 1) * HD, :] * SCALE        # [64, 768]
            wk = w_qkv[D + h * HD:D + (h + 1) * HD, :]        # [64, 768]
            for cc in range(NC_CC):
                wqk[:, jj * NC_CC + cc, 0:HD] = wq[:, cc * P:(cc + 1) * P].T
                wqk[:, jj * NC_CC + cc, HD:P] = wk[:, cc * P:(cc + 1) * P].T

        wv = np.empty((P, NC_CC, HPC * HD), dtype=np.float32)
        for jj in range(HPC):
            h = 3 * q4 + jj
            wvh = w_qkv[2 * D + h * HD:2 * D + (h + 1) * HD, :]  # [64, 768]
            for cc in range(NC_CC):
                wv[:, cc, jj * HD:(jj + 1) * HD] = wvh[:, cc * P:(cc + 1) * P].T

        # wfc[p, cc12, e] over 1536 gathered rows; zero rows for foreign ranks
        wfc_rows = np.zeros((NCORES * HPC * HD, D), dtype=np.float32)
        for s in range(NCORES):
            if s // 4 != g:
                continue
            for jj in range(HPC):
                h = 3 * (s % 4) + jj
                wfc_rows[s * HPC * HD + jj * HD:
                         s * HPC * HD + (jj + 1) * HD, :] = \
                    w_fc[:, h * HD:(h + 1) * HD].T
        wfc = wfc_rows.reshape(2 * NC_CC, P, D).transpose(1, 0, 2)

        maskb = (MASK_VAL * (padding_mask[g] > 0)).astype(np.float32)
        maskb = maskb.reshape(NC_KC, P).T.copy()  # [p, kc]

        bfc = np.tile(np.asarray(b_fc, dtype=np.float32)[None, :], (P, 1))

        in_maps.append({
            "x": x,
            "wqk": np.ascontiguousarray(wqk).astype(_BF16),
            "wv": np.ascontiguousarray(wv).astype(_BF16),
            "wfc": np.ascontiguousarray(wfc).astype(_BF16),
            "maskb": maskb,
            "bfc": bfc,
        })
    return in_maps


_CACHED_NC = None


def get_nc():
    global _CACHED_NC
    if _CACHED_NC is None:
        _CACHED_NC = build_nc()
    return _CACHED_NC


def kernel(inputs, padding_mask, w_qkv, w_fc, b_fc):
    inputs = np.asarray(inputs)
    padding_mask = np.asarray(padding_mask)
    w_qkv = np.asarray(w_qkv, dtype=np.float32)
    w_fc = np.asarray(w_fc, dtype=np.float32)
    b_fc = np.asarray(b_fc, dtype=np.float32)

    from concourse.bass_utils import run_bass_kernel_spmd

    nc = get_nc()
    in_maps = _prep_in_maps(inputs, padding_mask, w_qkv, w_fc, b_fc)
    res = run_bass_kernel_spmd(nc, in_maps, list(range(NCORES)))
    out = np.empty((B, N, D), dtype=np.float32)
    for c in range(NCORES):
        out[c // 4, (c % 4) * 512:(c % 4 + 1) * 512, :] = res.results[c]["out"]
    return out


# revision 20
# speedup vs baseline: 19.1688x; 1.0004x over previous
"""Distributed Trainium2 kernel for nn_Attention (B=2, N=2048, D=768, H=12).

Sharding: core c handles batch c//4 and head-triple c%4 (3 heads) for the
attention; the FC output projection is query-split (core c computes rows
(c%4)*512 .. +512 of its batch). The context tensor moves between the two
shardings with per-head 8-core AllToAlls of ctx^T column blocks; the FC
contracts over all 8 ranks' head-dims with rows of w_fc^T zeroed for the
other batch's ranks, which keeps the SPMD program identical on every core.

Compute dtype: bf16 matmul operands, fp32 PSUM accumulation and softmax.
The padding mask is folded into the exp bias (-1e7 per masked key), the
1/sqrt(hd) scale into the pre-transposed q weights, and the softmax
denominator rides the PV matmul as a 65th ones-column of V.
"""

import sys
import numpy as np

sys.path.insert(0, "/opt/trn_rl_repo")

import ml_dtypes

B, N, D, H, HD = 2, 2048, 768, 12, 64
P = 128
NCORES = 8
HPC = 3  # heads per core
NC_I4 = N // 512
NC_KC = N // P
NC_CC = D // P
SCALE = HD ** (-0.5)
MASK_VAL = -10000000.0

_BF16 = ml_dtypes.bfloat16


def _fix_multi_waits(nc):
    """walrus in this container accepts only ONE semaphore wait per
    instruction; hoist extra waits onto EventSemaphore carriers inserted
    immediately before, on the same engine (program order preserved)."""
    import bass_rust

    for b in nc.main_func.blocks:
        insts = b.instructions
        idx = 0
        while idx < len(insts):
            ins = insts[idx]
            si = ins.sync_info
            if si is None or len(si.on_wait) <= 1:
                idx += 1
                continue
            waits = list(si.on_wait)
            excess, keep = waits[:-1], waits[-1:]
            carriers = []
            for k, w in enumerate(excess):
                e = bass_rust.InstEventSemaphore(
                    name=f"{ins.name}_waitsplit_{k}", ins=[], outs=[]
                )
                e.engine = ins.engine
                esi = e.sync_info
                if esi is None:
                    esi = bass_rust.SyncInfo(on_wait=[], on_update=[])
                esi.on_wait = [w]
                e.sync_info = esi
                if ins.debug is not None:
                    e.debug = ins.debug
                carriers.append(e)
            si.on_wait = keep
            ins.sync_info = si
            for k, e in enumerate(carriers):
                insts.insert(idx + k, e)
            idx += len(carriers) + 1


def build_nc(variant="full"):
    import concourse.bass as bass
    import concourse.mybir as mybir
    import concourse.tile as tile

    BF16, F32 = mybir.dt.bfloat16, mybir.dt.float32
    AF = mybir.ActivationFunctionType
    ALU = mybir.AluOpType

    do_proj = variant in ("full", "nofc", "attn")
    do_attn = variant in ("full", "nofc", "attn")
    do_a2a = variant in ("full", "nofc")
    do_fc = variant in ("full",)
    do_x = variant != "empty"

    nc = bass.Bass()
    x_ext = nc.declare_dram_parameter("x", [N, D], BF16, isOutput=False)
    wqk_ext = nc.declare_dram_parameter("wqk", [P, HPC * NC_CC, P], BF16, isOutput=False)
    wv_ext = nc.declare_dram_parameter("wv", [P, NC_CC, HPC * HD], BF16, isOutput=False)
    wfc_ext = nc.declare_dram_parameter("wfc", [P, 2 * NC_CC, D], BF16, isOutput=False)
    maskb_ext = nc.declare_dram_parameter("maskb", [P, NC_KC], F32, isOutput=False)
    bfc_ext = nc.declare_dram_parameter("bfc", [P, D], F32, isOutput=False)
    out_ext = nc.declare_dram_parameter("out", [512, D], F32, isOutput=True)

    with tile.TileContext(nc) as tc:
        with (
            tc.tile_pool(name="persist", bufs=1) as persist,
            tc.tile_pool(name="stage", bufs=3) as stage,
            tc.tile_pool(name="pt", bufs=4) as ptp,
            tc.tile_pool(name="ctx", bufs=3) as ctxp,
            tc.tile_pool(name="outp", bufs=2) as outp,
            tc.tile_pool(name="ps", bufs=2, space="PSUM") as ps,
            tc.tile_pool(name="ps_ctx", bufs=4, space="PSUM") as ps_ctx,
            tc.tile_pool(name="dram", bufs=1, space="DRAM") as dram,
        ):
            # ---- persistent SBUF tensors
            xTq = [persist.tile([P, NC_CC, 512], BF16, name=f"xTq{q}", tag=f"xTq{q}")
                   for q in range(NC_I4)]  # x^T  [c, i] by seq quarter
            wqk = persist.tile([P, HPC * NC_CC, P], BF16)
            wv = persist.tile([P, NC_CC, HPC * HD], BF16)
            wfc = persist.tile([P, 2 * NC_CC, D], BF16)
            maskb = persist.tile([P, NC_KC], F32)
            bfc = persist.tile([P, D], F32)
            qT = [persist.tile([HD, N], BF16, name=f"qT{j}", tag=f"qT{j}")
                  for j in range(HPC)]
            kT = [persist.tile([HD, N], BF16, name=f"kT{j}", tag=f"kT{j}")
                  for j in range(HPC)]
            vv = persist.tile([P, NC_KC, HPC, HD + 1], BF16)  # V + ones col
            rec = persist.tile([1, N], F32)
            rbc = persist.tile([HD, N], F32)
            fcin = persist.tile([P, 2 * NC_CC, 512], BF16)

            nc.sync.dma_start(wqk[:], wqk_ext[:])
            nc.sync.dma_start(wv[:], wv_ext[:])
            nc.sync.dma_start(wfc[:], wfc_ext[:])
            nc.sync.dma_start(maskb[:], maskb_ext[:])
            nc.sync.dma_start(bfc[:], bfc_ext[:])
            nc.vector.memset(vv[:, :, :, HD:HD + 1], 1.0)

            # ---- DRAM internals
            a2a_in = [dram.tile([NCORES, HD, 512], BF16, name=f"a2ai{j}")
                      for j in range(HPC)]
            a2a_out = [dram.tile([NCORES, HD, 512], BF16, name=f"a2ao{j}")
                       for j in range(HPC)]
            rb_dram = dram.tile([HD, N], F32)

            # ---- phase 1: XBAR-transpose x (already bf16) from DRAM,
            #      split by sequence quarter so projections pipeline
            for q in range(NC_I4 if do_x else 0):
                for cc in range(NC_CC):
                    nc.sync.dma_start_transpose(
                        xTq[q][:, cc, :],
                        x_ext[q * 512:(q + 1) * 512, cc * P:(cc + 1) * P])

            # ---- phase 2: q/k projections (pair-stacked, pre-scaled q)
            for j in range(HPC if do_proj else 0):
                for i4 in range(NC_I4):
                    pqk = ps.tile([P, 512], mybir.dt.float32, tag="ps", name="pqk")
                    for cc in range(NC_CC):
                        nc.tensor.matmul(
                            pqk[:],
                            lhsT=wqk[:, j * NC_CC + cc, :],
                            rhs=xTq[i4][:, cc, :],
                            start=(cc == 0),
                            stop=(cc == NC_CC - 1),
                        )
                    sl = slice(i4 * 512, (i4 + 1) * 512)
                    nc.vector.tensor_copy(qT[j][:, sl], pqk[0:HD, :])
                    nc.vector.tensor_copy(kT[j][:, sl], pqk[HD:P, :])

            # ---- phase 3: v projection (natural layout)
            for kc in range(NC_KC if do_proj else 0):
                pv = ps.tile([P, 512], mybir.dt.float32, tag="ps", name="pv")
                for cc in range(NC_CC):
                    nc.tensor.matmul(
                        pv[:, 0:HPC * HD],
                        lhsT=xTq[kc // 4][:, cc, (kc % 4) * P:(kc % 4 + 1) * P],
                        rhs=wv[:, cc, :],
                        start=(cc == 0),
                        stop=(cc == NC_CC - 1),
                    )
                nc.vector.tensor_copy(
                    vv[:, kc, :, 0:HD],
                    pv[:, 0:HPC * HD].rearrange("p (j d) -> p j d", j=HPC),
                )

            # ---- phase 4: attention per head (1024-wide softmax tiles)
            for j in range(HPC if do_attn else 0):
                pctx = [ps_ctx.tile([HD + 1, 512], mybir.dt.float32,
                                    tag="pctx", name=f"pctx{i4}")
                        for i4 in range(NC_I4)]
                for kc in range(NC_KC):
                    for ih in range(2):
                        pss = ps.tile([P, 1024], mybir.dt.float32, tag="ps",
                                      name="pss")
                        for i2 in range(2):
                            i4 = ih * 2 + i2
                            nc.tensor.matmul(
                                pss[:, i2 * 512:(i2 + 1) * 512],
                                lhsT=kT[j][:, kc * P:(kc + 1) * P],
                                rhs=qT[j][:, i4 * 512:(i4 + 1) * 512],
                                start=True,
                                stop=True,
                            )
                        pT = ptp.tile([P, 1024], BF16, tag="pT")
                        nc.scalar.activation(
                            pT[:], pss[:], AF.Exp,
                            bias=maskb[:, kc:kc + 1], scale=1.0,
                        )
                        for i2 in range(2):
                            i4 = ih * 2 + i2
                            nc.tensor.matmul(
                                pctx[i4][:],
                                lhsT=vv[:, kc, j, :],
                                rhs=pT[:, i2 * 512:(i2 + 1) * 512],
                                start=(kc == 0),
                                stop=(kc == NC_KC - 1),
                            )
                # quick-release PSUM: copy unnormalized ctx + denom to SBUF
                ub = ctxp.tile([HD + 1, NC_I4, 512], mybir.dt.float32, tag="ub",
                               name="ub")
                for i4 in range(NC_I4):
                    nc.vector.tensor_copy(ub[:, i4, :], pctx[i4][:])
                # reciprocal -> DRAM-doubling broadcast
                nc.vector.reciprocal(rec[0:1, :],
                                     ub[HD:HD + 1, :, :].rearrange("e q n -> e (q n)"))
                nc.sync.dma_start(rb_dram[0:1, :], rec[:])
                n = 1
                while n < HD:
                    m = min(n, HD - n)
                    nc.sync.dma_start(rb_dram[n:n + m, :], rb_dram[0:m, :])
                    n += m
                nc.sync.dma_start(rbc[:], rb_dram[:])
                # normalize into one per-head tile, ship to A2A input
                cst = ctxp.tile([HD, NC_I4, 512], BF16, tag="cst")
                nc.vector.tensor_tensor(
                    cst[:].rearrange("d q n -> d (q n)"),
                    ub[0:HD, :, :].rearrange("d q n -> d (q n)"),
                    rbc[:],
                    ALU.mult,
                )
                for i4 in range(NC_I4):
                    nc.sync.dma_start(
                        a2a_in[j][i4, :, :], cst[:, i4, :])
                    nc.sync.dma_start(
                        a2a_in[j][i4 + 4, :, :], cst[:, i4, :])
                if do_a2a:
                    nc.gpsimd.collective_compute(
                        "AllToAll",
                        mybir.AluOpType.bypass,
                        replica_groups=[list(range(NCORES))],
                        ins=[a2a_in[j].opt()],
                        outs=[a2a_out[j].opt()],
                    )

            # ---- phase 6: FC over 1536-row gathered ctx^T, add bias
            if not do_fc:
                ob0 = outp.tile([P, D], F32, tag="ob", name="ob0")
                nc.vector.memset(ob0[:], 0.0)
                for i4 in range(NC_I4):
                    nc.sync.dma_start(out_ext[i4 * P:(i4 + 1) * P, :], ob0[:])
            # gather: global contraction row r = s*192 + j*64 + dd comes from
            # a2a_out[j][s, dd, :]; per 64-row granule.
            for gr in range(2 * NC_CC * 2 if do_fc else 0):
                r = gr * HD
                s_, j_ = r // (HPC * HD), (r % (HPC * HD)) // HD
                cc, half = gr // 2, gr % 2
                nc.gpsimd.dma_start(
                    fcin[half * HD:(half + 1) * HD, cc, :],
                    a2a_out[j_][s_, :, :])
            cc_order = sorted(range(2 * NC_CC),
                              key=lambda cc: max((cc * P + half * HD) % (HPC * HD) // HD
                                                 for half in range(2)))
            for i4 in range(NC_I4 if do_fc else 0):
                pfa = ps.tile([P, 512], mybir.dt.float32, tag="ps", name="pfa")
                pfb = ps.tile([P, 512], mybir.dt.float32, tag="ps", name="pfb")
                for ci, cc in enumerate(cc_order):
                    lhsT = fcin[:, cc, i4 * P:(i4 + 1) * P]
                    nc.tensor.matmul(
                        pfa[:], lhsT=lhsT, rhs=wfc[:, cc, 0:512],
                        start=(ci == 0), stop=(ci == 2 * NC_CC - 1))
                    nc.tensor.matmul(
                        pfb[:, 0:D - 512], lhsT=lhsT, rhs=wfc[:, cc, 512:D],
                        start=(ci == 0), stop=(ci == 2 * NC_CC - 1))
                ob = outp.tile([P, D], F32, tag="ob", name="ob")
                nc.vector.tensor_tensor(ob[:, 0:512], pfa[:], bfc[:, 0:512], ALU.add)
                nc.vector.tensor_tensor(ob[:, 512:D], pfb[:, 0:D - 512],
                                        bfc[:, 512:D], ALU.add)
                nc.sync.dma_start(out_ext[i4 * P:(i4 + 1) * P, :], ob[:])

    _fix_multi_waits(nc)
    return nc


def _prep_in_maps(inputs, padding_mask, w_qkv, w_fc, b_fc):
    in_maps = []
    for c in range(NCORES):
        g, q4 = c // 4, c % 4
        x = np.ascontiguousarray(inputs[g], dtype=np.float32).astype(_BF16)

        # wqk[p, j*6+cc, m]: m<64 -> scaled WqT, else WkT
        wqk = np.empty((P, HPC * NC_CC, P), dtype=np.float32)
        for jj in range(HPC):
            h = 3 * q4 + jj
            wq = w_qkv[h * HD:(h + 1) * HD, :] * SCALE        # [64, 768]
            wk = w_qkv[D + h * HD:D + (h + 1) * HD, :]        # [64, 768]
            for cc in range(NC_CC):
                wqk[:, jj * NC_CC + cc, 0:HD] = wq[:, cc * P:(cc + 1) * P].T
                wqk[:, jj * NC_CC + cc, HD:P] = wk[:, cc * P:(cc + 1) * P].T

        wv = np.empty((P, NC_CC, HPC * HD), dtype=np.float32)
        for jj in range(HPC):
            h = 3 * q4 + jj
            wvh = w_qkv[2 * D + h * HD:2 * D + (h + 1) * HD, :]  # [64, 768]
            for cc in range(NC_CC):
                wv[:, cc, jj * HD:(jj + 1) * HD] = wvh[:, cc * P:(cc + 1) * P].T

        # wfc[p, cc12, e] over 1536 gathered rows; zero rows for foreign ranks
        wfc_rows = np.zeros((NCORES * HPC * HD, D), dtype=np.float32)
        for s in range(NCORES):
            if s // 4 != g:
                continue
            for jj in range(HPC):
                h = 3 * (s % 4) + jj
                wfc_rows[s * HPC * HD + jj * HD:
                         s * HPC * HD + (jj + 1) * HD, :] = \
                    w_fc[:, h * HD:(h + 1) * HD].T
        wfc = wfc_rows.reshape(2 * NC_CC, P, D).transpose(1, 0, 2)

        maskb = (MASK_VAL * (padding_mask[g] > 0)).astype(np.float32)
        maskb = maskb.reshape(NC_KC, P).T.copy()  # [p, kc]

        bfc = np.tile(np.asarray(b_fc, dtype=np.float32)[None, :], (P, 1))

        in_maps.append({
            "x": x,
            "wqk": np.ascontiguousarray(wqk).astype(_BF16),
            "wv": np.ascontiguousarray(wv).astype(_BF16),
            "wfc": np.ascontiguousarray(wfc).astype(_BF16),
            "maskb": maskb,
            "bfc": bfc,
        })
    return in_maps


_CACHED_NC = None


def get_nc():
    global _CACHED_NC
    if _CACHED_NC is None:
        _CACHED_NC = build_nc()
    return _CACHED_NC


def kernel(inputs, padding_mask, w_qkv, w_fc, b_fc):
    inputs = np.asarray(inputs)
    padding_mask = np.asarray(padding_mask)
    w_qkv = np.asarray(w_qkv, dtype=np.float32)
    w_fc = np.asarray(w_fc, dtype=np.float32)
    b_fc = np.asarray(b_fc, dtype=np.float32)

    from concourse.bass_utils import run_bass_kernel_spmd

    nc = get_nc()
    in_maps = _prep_in_maps(inputs, padding_mask, w_qkv, w_fc, b_fc)
    res = run_bass_kernel_spmd(nc, in_maps, list(range(NCORES)))
    out = np.empty((B, N, D), dtype=np.float32)
    for c in range(NCORES):
        out[c // 4, (c % 4) * 512:(c % 4 + 1) * 512, :] = res.results[c]["out"]
    return out


# revision 30
# speedup vs baseline: 20.4436x; 1.0665x over previous
"""Distributed Trainium2 kernel for nn_Attention (B=2, N=2048, D=768, H=12).

Sharding: core c handles batch c//4 and head-triple c%4 (3 heads) for the
attention; the FC output projection is query-split (core c computes rows
(c%4)*512 .. +512 of its batch). The context tensor moves between the two
shardings with per-head 8-core AllToAlls of ctx^T column blocks; the FC
contracts over all 8 ranks' head-dims with rows of w_fc^T zeroed for the
other batch's ranks, which keeps the SPMD program identical on every core.

Compute dtype: bf16 matmul operands, fp32 PSUM accumulation and softmax.
The padding mask is folded into the exp bias (-1e7 per masked key), the
1/sqrt(hd) scale into the pre-transposed q weights, and the softmax
denominator rides the PV matmul as a 65th ones-column of V.
"""

import sys
import numpy as np

sys.path.insert(0, "/opt/trn_rl_repo")

import ml_dtypes

B, N, D, H, HD = 2, 2048, 768, 12, 64
P = 128
NCORES = 8
HPC = 3  # heads per core
NC_I4 = N // 512
NC_KC = N // P
NC_CC = D // P
SCALE = HD ** (-0.5)
MASK_VAL = -10000000.0

_BF16 = ml_dtypes.bfloat16


def _fix_multi_waits(nc):
    """walrus in this container accepts only ONE semaphore wait per
    instruction; hoist extra waits onto EventSemaphore carriers inserted
    immediately before, on the same engine (program order preserved)."""
    import bass_rust

    for b in nc.main_func.blocks:
        insts = b.instructions
        idx = 0
        while idx < len(insts):
            ins = insts[idx]
            si = ins.sync_info
            if si is None or len(si.on_wait) <= 1:
                idx += 1
                continue
            waits = list(si.on_wait)
            excess, keep = waits[:-1], waits[-1:]
            carriers = []
            for k, w in enumerate(excess):
                e = bass_rust.InstEventSemaphore(
                    name=f"{ins.name}_waitsplit_{k}", ins=[], outs=[]
                )
                e.engine = ins.engine
                esi = e.sync_info
                if esi is None:
                    esi = bass_rust.SyncInfo(on_wait=[], on_update=[])
                esi.on_wait = [w]
                e.sync_info = esi
                if ins.debug is not None:
                    e.debug = ins.debug
                carriers.append(e)
            si.on_wait = keep
            ins.sync_info = si
            for k, e in enumerate(carriers):
                insts.insert(idx + k, e)
            idx += len(carriers) + 1


def build_nc(variant="full"):
    import concourse.bass as bass
    import concourse.mybir as mybir
    import concourse.tile as tile

    BF16, F32 = mybir.dt.bfloat16, mybir.dt.float32
    AF = mybir.ActivationFunctionType
    ALU = mybir.AluOpType

    do_proj = variant in ("full", "nofc", "attn")
    do_attn = variant in ("full", "nofc", "attn")
    do_a2a = variant in ("full", "nofc")
    do_fc = variant in ("full",)
    do_x = variant != "empty"

    nc = bass.Bass()
    x_ext = nc.declare_dram_parameter("x", [N, D], BF16, isOutput=False)
    wqk_ext = nc.declare_dram_parameter("wqk", [P, HPC * NC_CC, P], BF16, isOutput=False)
    wv_ext = nc.declare_dram_parameter("wv", [P, NC_CC, HPC * HD], BF16, isOutput=False)
    wfc_ext = nc.declare_dram_parameter("wfc", [P, 2 * NC_CC, D], BF16, isOutput=False)
    maskb_ext = nc.declare_dram_parameter("maskb", [P, NC_KC], F32, isOutput=False)
    bfc_ext = nc.declare_dram_parameter("bfc", [P, D], F32, isOutput=False)
    out_ext = nc.declare_dram_parameter("out", [512, D], F32, isOutput=True)

    with tile.TileContext(nc) as tc:
        with (
            tc.tile_pool(name="persist", bufs=1) as persist,
            tc.tile_pool(name="stage", bufs=3) as stage,
            tc.tile_pool(name="pt", bufs=6) as ptp,
            tc.tile_pool(name="ctx", bufs=4) as ctxp,
            tc.tile_pool(name="outp", bufs=2) as outp,
            tc.tile_pool(name="ps", bufs=2, space="PSUM") as ps,
            tc.tile_pool(name="ps_ctx", bufs=4, space="PSUM") as ps_ctx,
            tc.tile_pool(name="dram", bufs=1, space="DRAM") as dram,
        ):
            # ---- persistent SBUF tensors
            xTq = [persist.tile([P, NC_CC, 512], BF16, name=f"xTq{q}", tag=f"xTq{q}")
                   for q in range(NC_I4)]  # x^T  [c, i] by seq quarter
            wqk = persist.tile([P, HPC * NC_CC, P], BF16)
            wv = persist.tile([P, NC_CC, HPC * HD], BF16)
            wfc = persist.tile([P, 2 * NC_CC, D], BF16)
            maskb = persist.tile([P, NC_KC], F32)
            bfc = persist.tile([P, D], F32)
            qT = [persist.tile([HD, N], BF16, name=f"qT{j}", tag=f"qT{j}")
                  for j in range(HPC)]
            kT = [persist.tile([HD, N], BF16, name=f"kT{j}", tag=f"kT{j}")
                  for j in range(HPC)]
            vvq = [persist.tile([P, 4, HPC, HD + 1], BF16, name=f"vvq{q}",
                                tag=f"vvq{q}")
                   for q in range(NC_I4)]  # V + ones col, by key quarter
            recb = persist.tile([P, N], F32)   # row 0 = reciprocal denoms
            e0 = persist.tile([P, HD], F32)    # ones in row 0, else zero
            fcin_j = [persist.tile([P, 4, 512], BF16, name=f"fcin{j}",
                                   tag=f"fcin{j}")
                      for j in range(HPC)]
            acc = [persist.tile([P, D], F32, name=f"acc{i4}", tag=f"acc{i4}")
                   for i4 in range(NC_I4)]

            nc.sync.dma_start(wqk[:], wqk_ext[:])
            nc.sync.dma_start(wv[:], wv_ext[:])
            nc.gpsimd.dma_start(wfc[:], wfc_ext[:])
            nc.sync.dma_start(maskb[:], maskb_ext[:])
            nc.gpsimd.dma_start(bfc[:], bfc_ext[:])
            for q in range(NC_I4):
                nc.vector.memset(vvq[q][:, :, :, HD:HD + 1], 1.0)
            nc.vector.memset(recb[:], 0.0)
            nc.vector.memset(e0[:], 0.0)
            nc.vector.memset(e0[0:1, :], 1.0)

            # ---- DRAM internals
            a2a_in = [dram.tile([NCORES, HD, 512], BF16, name=f"a2ai{j}")
                      for j in range(HPC)]
            a2a_out = [dram.tile([NCORES, HD, 512], BF16, name=f"a2ao{j}")
                       for j in range(HPC)]

            # ---- phase 1: XBAR-transpose x (already bf16) from DRAM,
            #      split by sequence quarter so projections pipeline
            for q in range(NC_I4 if do_x else 0):
                for cc in range(NC_CC):
                    nc.sync.dma_start_transpose(
                        xTq[q][:, cc, :],
                        x_ext[q * 512:(q + 1) * 512, cc * P:(cc + 1) * P])

            # ---- phase 2: q/k projections (pair-stacked, pre-scaled q)
            for j in range(HPC if do_proj else 0):
                for i4 in range(NC_I4):
                    pqk = ps.tile([P, 512], mybir.dt.float32, tag="ps", name="pqk")
                    for cc in range(NC_CC):
                        nc.tensor.matmul(
                            pqk[:],
                            lhsT=wqk[:, j * NC_CC + cc, :],
                            rhs=xTq[i4][:, cc, :],
                            start=(cc == 0),
                            stop=(cc == NC_CC - 1),
                        )
                    sl = slice(i4 * 512, (i4 + 1) * 512)
                    nc.vector.tensor_copy(qT[j][:, sl], pqk[0:HD, :])
                    nc.vector.tensor_copy(kT[j][:, sl], pqk[HD:P, :])

            # ---- phase 3: v projection (natural layout)
            for kc in range(NC_KC if do_proj else 0):
                pv = ps.tile([P, 512], mybir.dt.float32, tag="ps", name="pv")
                for cc in range(NC_CC):
                    nc.tensor.matmul(
                        pv[:, 0:HPC * HD],
                        lhsT=xTq[kc // 4][:, cc, (kc % 4) * P:(kc % 4 + 1) * P],
                        rhs=wv[:, cc, :],
                        start=(cc == 0),
                        stop=(cc == NC_CC - 1),
                    )
                nc.vector.tensor_copy(
                    vvq[kc // 4][:, kc % 4, :, 0:HD],
                    pv[:, 0:HPC * HD].rearrange("p (j d) -> p j d", j=HPC),
                )

            # ---- phase 4: attention per head (1024-wide softmax tiles).
            # Emission order interleaves each head's normalize/ship block
            # under the NEXT head's attention so the PE-broadcast psum tiles
            # never starve the softmax pipeline at head boundaries.
            ubs = [None] * HPC

            def att_block(j):
                pctx = {}
                for kc in range(NC_KC):
                    for ih in range(2):
                        pss = ps.tile([P, 1024], mybir.dt.float32, tag="ps",
                                      name="pss")
                        for i2 in range(2):
                            i4 = ih * 2 + i2
                            nc.tensor.matmul(
                                pss[:, i2 * 512:(i2 + 1) * 512],
                                lhsT=kT[j][:, kc * P:(kc + 1) * P],
                                rhs=qT[j][:, i4 * 512:(i4 + 1) * 512],
                                start=True,
                                stop=True,
                            )
                        pT = ptp.tile([P, 1024], BF16, tag="pT")
                        nc.scalar.activation(
                            pT[:], pss[:], AF.Exp,
                            bias=maskb[:, kc:kc + 1], scale=1.0,
                        )
                        for i2 in range(2):
                            i4 = ih * 2 + i2
                            if i4 not in pctx:
                                pctx[i4] = ps_ctx.tile(
                                    [HD + 1, 512], mybir.dt.float32,
                                    tag="pctx", name=f"pctx{i4}")
                            nc.tensor.matmul(
                                pctx[i4][:],
                                lhsT=vvq[kc // 4][:, kc % 4, j, :],
                                rhs=pT[:, i2 * 512:(i2 + 1) * 512],
                                start=(kc == 0),
                                stop=(kc == NC_KC - 1),
                            )
                # quick-release PSUM: copy unnormalized ctx + denom to SBUF
                ub = ctxp.tile([HD + 1, NC_I4, 512], mybir.dt.float32,
                               tag="ub", name="ub")
                for i4 in range(NC_I4):
                    nc.vector.tensor_copy(ub[:, i4, :], pctx[i4][:])
                ubs[j] = ub

            def fin_block(j):
                ub = ubs[j]
                # reciprocal -> PE broadcast (e0 outer product, exact f32)
                nc.vector.reciprocal(
                    recb[0:1, :],
                    ub[HD:HD + 1, :, :].rearrange("e q n -> e (q n)"))
                cst = ctxp.tile([HD, NC_I4, 512], BF16, tag="cst")
                for ih in range(2):
                    prb = ps.tile([HD, 1024], mybir.dt.float32, tag="ps",
                                  name="prb")
                    for i2 in range(2):
                        i4 = ih * 2 + i2
                        nc.tensor.matmul(
                            prb[:, i2 * 512:(i2 + 1) * 512],
                            lhsT=e0[:],
                            rhs=recb[:, i4 * 512:(i4 + 1) * 512],
                            start=True, stop=True)
                        nc.vector.tensor_tensor(
                            cst[:, i4, :],
                            ub[0:HD, i4, :],
                            prb[:, i2 * 512:(i2 + 1) * 512],
                            ALU.mult,
                        )
                nc.sync.dma_start(
                    a2a_in[j][0:4, :, :].rearrange("q d n -> d q n"), cst[:])
                nc.sync.dma_start(
                    a2a_in[j][4:8, :, :].rearrange("q d n -> d q n"), cst[:])
                if do_a2a:
                    nc.gpsimd.collective_compute(
                        "AllToAll",
                        mybir.AluOpType.bypass,
                        replica_groups=[list(range(NCORES))],
                        ins=[a2a_in[j].opt()],
                        outs=[a2a_out[j].opt()],
                    )
                if do_fc:
                    # gather this head's granules (row s*64+dd of fcin_j):
                    # dst partition p=(h,dd) <- src shard cl*2+h
                    nc.sync.dma_start(
                        fcin_j[j][:],
                        a2a_out[j][:].rearrange("(cl h) dd n -> (h dd) cl n",
                                                h=2))

            # ---- FC: per-head partials accumulated in SBUF (j-major rows)
            def fc_block(j):
                for i4 in range(NC_I4):
                    pf = ps.tile([P, 1024], mybir.dt.float32, tag="ps",
                                 name="pf")
                    for cl in range(4):
                        lhsT = fcin_j[j][:, cl, i4 * P:(i4 + 1) * P]
                        nc.tensor.matmul(
                            pf[:, 0:512], lhsT=lhsT,
                            rhs=wfc[:, j * 4 + cl, 0:512],
                            start=(cl == 0), stop=(cl == 3))
                        nc.tensor.matmul(
                            pf[:, 512:512 + (D - 512)], lhsT=lhsT,
                            rhs=wfc[:, j * 4 + cl, 512:D],
                            start=(cl == 0), stop=(cl == 3))
                    if j == 0:
                        nc.vector.tensor_tensor(
                            acc[i4][:, 0:512], pf[:, 0:512],
                            bfc[:, 0:512], ALU.add)
                        nc.vector.tensor_tensor(
                            acc[i4][:, 512:D], pf[:, 512:512 + (D - 512)],
                            bfc[:, 512:D], ALU.add)
                    else:
                        nc.vector.tensor_tensor(
                            acc[i4][:, 0:512], acc[i4][:, 0:512],
                            pf[:, 0:512], ALU.add)
                        nc.vector.tensor_tensor(
                            acc[i4][:, 512:D], acc[i4][:, 512:D],
                            pf[:, 512:512 + (D - 512)], ALU.add)


            if do_attn:
                for j in range(HPC):
                    att_block(j)
                    fin_block(j)
            if do_fc:
                for j in range(HPC):
                    fc_block(j)

            # ---- outputs
            if do_fc:
                for i4 in range(NC_I4):
                    nc.sync.dma_start(out_ext[i4 * P:(i4 + 1) * P, :],
                                      acc[i4][:])
            else:
                ob0 = outp.tile([P, D], F32, tag="ob", name="ob0")
                nc.vector.memset(ob0[:], 0.0)
                for i4 in range(NC_I4):
                    nc.sync.dma_start(out_ext[i4 * P:(i4 + 1) * P, :], ob0[:])

    _fix_multi_waits(nc)
    return nc


def _prep_in_maps(inputs, padding_mask, w_qkv, w_fc, b_fc):
    in_maps = []
    for c in range(NCORES):
        g, q4 = c // 4, c % 4
        x = np.ascontiguousarray(inputs[g], dtype=np.float32).astype(_BF16)

        # wqk[p, j*6+cc, m]: m<64 -> scaled WqT, else WkT
        wqk = np.empty((P, HPC * NC_CC, P), dtype=np.float32)
        for jj in range(HPC):
            h = 3 * q4 + jj
            wq = w_qkv[h * HD:(h + 1) * HD, :] * SCALE        # [64, 768]
            wk = w_qkv[D + h * HD:D + (h + 1) * HD, :]        # [64, 768]
            for cc in range(NC_CC):
                wqk[:, jj * NC_CC + cc, 0:HD] = wq[:, cc * P:(cc + 1) * P].T
                wqk[:, jj * NC_CC + cc, HD:P] = wk[:, cc * P:(cc + 1) * P].T

        wv = np.empty((P, NC_CC, HPC * HD), dtype=np.float32)
        for jj in range(HPC):
            h = 3 * q4 + jj
            wvh = w_qkv[2 * D + h * HD:2 * D + (h + 1) * HD, :]  # [64, 768]
            for cc in range(NC_CC):
                wv[:, cc, jj * HD:(jj + 1) * HD] = wvh[:, cc * P:(cc + 1) * P].T

        # wfc[p, cc12, e]: j-major gathered rows (r = j*512 + s*64 + dd);
        # zero rows for the other batch's ranks
        wfc_rows = np.zeros((NCORES * HPC * HD, D), dtype=np.float32)
        for jj in range(HPC):
            for s in range(NCORES):
                if s // 4 != g:
                    continue
                h = 3 * (s % 4) + jj
                base = jj * NCORES * HD + s * HD
                wfc_rows[base:base + HD, :] = w_fc[:, h * HD:(h + 1) * HD].T
        wfc = wfc_rows.reshape(2 * NC_CC, P, D).transpose(1, 0, 2)

        maskb = (MASK_VAL * (padding_mask[g] > 0)).astype(np.float32)
        maskb = maskb.reshape(NC_KC, P).T.copy()  # [p, kc]

        bfc = np.tile(np.asarray(b_fc, dtype=np.float32)[None, :], (P, 1))

        in_maps.append({
            "x": x,
            "wqk": np.ascontiguousarray(wqk).astype(_BF16),
            "wv": np.ascontiguousarray(wv).astype(_BF16),
            "wfc": np.ascontiguousarray(wfc).astype(_BF16),
            "maskb": maskb,
            "bfc": bfc,
        })
    return in_maps


_CACHED_NC = None


def get_nc():
    global _CACHED_NC
    if _CACHED_NC is None:
        _CACHED_NC = build_nc()
    return _CACHED_NC


def kernel(inputs, padding_mask, w_qkv, w_fc, b_fc):
    inputs = np.asarray(inputs)
    padding_mask = np.asarray(padding_mask)
    w_qkv = np.asarray(w_qkv, dtype=np.float32)
    w_fc = np.asarray(w_fc, dtype=np.float32)
    b_fc = np.asarray(b_fc, dtype=np.float32)

    from concourse.bass_utils import run_bass_kernel_spmd

    nc = get_nc()
    in_maps = _prep_in_maps(inputs, padding_mask, w_qkv, w_fc, b_fc)
    res = run_bass_kernel_spmd(nc, in_maps, list(range(NCORES)))
    out = np.empty((B, N, D), dtype=np.float32)
    for c in range(NCORES):
        out[c // 4, (c % 4) * 512:(c % 4 + 1) * 512, :] = res.results[c]["out"]
    return out
